# revision 7
# baseline (speedup 1.0000x reference)
"""Gaussian blur 31x31 depthwise conv (reflect pad) on 8 trn2 NeuronCores.

Strategy:
  - Pure data parallel: 32 images -> 4 per core; each core handles 12 planes
    (4 images x 3 channels) of 512x512 f32.
  - The 31x31 kernel is separable (rank-1): factor via SVD into vertical /
    horizontal 1D taps on the host.
  - Each 1D conv (with reflection fold) is a banded matmul on the TensorEngine:
    out_block[M,512] = lhsT.T @ x_rows[K,512], where lhsT is a [K,M] slice of
    the banded-with-reflection conv matrix. Output blocks of BS=128-2*R rows
    need K = BS+2R = 128 input rows -> exactly one matmul per block.
  - The horizontal pass runs in the transposed domain; transposes are done on
    the TensorEngine (identity matmul), sliced in the free dim so the halo'd
    row-tiles of the transposed plane are produced directly.
"""

import numpy as np

H = W = 512
N_CORES = 8
IMG_PER_CORE = 4
CH = 3
NPLANE = IMG_PER_CORE * CH  # 12 planes per core

_cache = {}


def _factor_weight(weight):
    """Per-channel rank-1 factorization: w[c,0] = outer(kv, kh)."""
    kvs, khs = [], []
    for c in range(weight.shape[0]):
        k2 = weight[c, 0].astype(np.float64)
        u, s, vt = np.linalg.svd(k2)
        kv = u[:, 0] * np.sqrt(s[0])
        kh = vt[0] * np.sqrt(s[0])
        if kv.sum() < 0:
            kv, kh = -kv, -kh
        thr = 1e-12 * max(np.abs(kv).max(), np.abs(kh).max())
        kv[np.abs(kv) < thr] = 0.0
        kh[np.abs(kh) < thr] = 0.0
        kvs.append(kv)
        khs.append(kh)
    return kvs, khs


def _conv_matrix(k1):
    """C (512x512) such that out = C @ x for 1D conv with 'reflect' padding."""
    n = len(k1)
    r = n // 2
    C = np.zeros((H, H), dtype=np.float64)
    for j in range(-r, r + 1):
        w = k1[j + r]
        if w == 0.0:
            continue
        for o in range(H):
            t = o + j
            if t < 0:
                t = -t
            elif t > H - 1:
                t = 2 * (H - 1) - t
            C[o, t] += w
    return C


def _radius(k1):
    nz = np.nonzero(k1)[0]
    c = len(k1) // 2
    return int(max(nz.max() - c, c - nz.min())) if len(nz) else 0


def _blocks(radius):
    """Output row blocks with input row ranges (band support incl. reflection)."""
    bs = (128 - 2 * radius) // 32 * 32
    blocks = []
    o0 = 0
    while o0 < H:
        o1 = min(H, o0 + bs)
        i0 = max(0, o0 - radius)
        i1 = min(H, o1 + radius)
        blocks.append((o0, o1, i0, i1))
        o0 = o1
    return blocks


def _seg128(o0, o1):
    """Split global partition-row range into per-128-tile segments."""
    segs = []
    p = o0
    while p < o1:
        j = p // 128
        hi = min(o1, (j + 1) * 128)
        segs.append((j, p - j * 128, p - o0, hi - p))
        p = hi
    return segs


def _build_program(n_v, n_h, ch2v, ch2h, blocks, dt_mm):
    import concourse.bacc as bacc
    import concourse.mybir as mybir
    import concourse.tile as tile

    f32 = mybir.dt.float32
    nc = bacc.Bacc("TRN2", target_bir_lowering=False, debug=False,
                   num_devices=N_CORES)

    x_d = nc.dram_tensor("x", (NPLANE, H, W), f32, kind="ExternalInput")
    o_d = nc.dram_tensor("out", (NPLANE, H, W), f32, kind="ExternalOutput")
    id_d = nc.dram_tensor("ident", (128, 128), f32, kind="ExternalInput")
    lv_d = [[nc.dram_tensor(f"lv{s}_{b}", (i1 - i0, o1 - o0), f32,
                            kind="ExternalInput")
             for b, (o0, o1, i0, i1) in enumerate(blocks)] for s in range(n_v)]
    lh_d = [[nc.dram_tensor(f"lh{s}_{b}", (i1 - i0, o1 - o0), f32,
                            kind="ExternalInput")
             for b, (o0, o1, i0, i1) in enumerate(blocks)] for s in range(n_h)]

    xa, oa, ida = x_d.ap(), o_d.ap(), id_d.ap()
    nb = len(blocks)

    with tile.TileContext(nc) as tc:
        with (
            tc.tile_pool(name="const", bufs=1) as cpool,
            tc.tile_pool(name="xv", bufs=2) as xv_pool,
            tc.tile_pool(name="t1", bufs=2) as t1_pool,
            tc.tile_pool(name="xh", bufs=2) as xh_pool,
            tc.tile_pool(name="t2", bufs=2) as t2_pool,
            tc.tile_pool(name="ot", bufs=2) as ot_pool,
            tc.tile_pool(name="psA", bufs=2, space="PSUM") as psA,
            tc.tile_pool(name="psB", bufs=2, space="PSUM") as psB,
            tc.tile_pool(name="psC", bufs=2, space="PSUM") as psC,
            tc.tile_pool(name="psD", bufs=2, space="PSUM") as psD,
        ):
            ident = cpool.tile([128, 128], dt_mm, tag="ident")
            nc.sync.dma_start(ident[:], ida[:])
            lv = [[cpool.tile([i1 - i0, o1 - o0], dt_mm, tag=f"lv{s}_{b}",
                              name=f"lv{s}_{b}_t")
                   for b, (o0, o1, i0, i1) in enumerate(blocks)]
                  for s in range(n_v)]
            lh = [[cpool.tile([i1 - i0, o1 - o0], dt_mm, tag=f"lh{s}_{b}",
                              name=f"lh{s}_{b}_t")
                   for b, (o0, o1, i0, i1) in enumerate(blocks)]
                  for s in range(n_h)]
            for s in range(n_v):
                for b in range(nb):
                    nc.sync.dma_start(lv[s][b][:], lv_d[s][b].ap()[:])
            for s in range(n_h):
                for b in range(nb):
                    nc.sync.dma_start(lh[s][b][:], lh_d[s][b].ap()[:])

            cnt = [0]

            def copy(out, in_):
                eng = (nc.vector.tensor_copy, nc.scalar.copy)[cnt[0] % 2]
                eng(out, in_)
                cnt[0] += 1

            def copy_seg(dst, psrc, o0, o1):
                # Engine APs with nonzero partition start may span at most 32
                # partitions (start must be a multiple of 32); start-0 APs may
                # span all 128.  Block edges are multiples of 32, so chunking
                # into 32-row pieces satisfies both rules.
                for (j, dp, sp, rows) in _seg128(o0, o1):
                    if dp == 0 and sp == 0:
                        copy(dst[:rows, j, :], psrc[:rows, :])
                    else:
                        for c0 in range(0, rows, 32):
                            n = min(32, rows - c0)
                            copy(dst[dp + c0: dp + c0 + n, j, :],
                                 psrc[sp + c0: sp + c0 + n, :])

            for p in range(NPLANE):
                sv, sh = ch2v[p % CH], ch2h[p % CH]

                # Stage A: load halo'd row tiles, vertical banded matmul.
                # Block edges are multiples of 32, so every PSUM->SBUF copy
                # below has start/size multiples of 32 (ACT/DVE constraint);
                # t1/t2 are 4x(128,W) row tiles, transposes always full-128.
                xv = xv_pool.tile([128, nb, W], dt_mm, tag="xv")
                for b, (o0, o1, i0, i1) in enumerate(blocks):
                    nc.sync.dma_start(xv[: i1 - i0, b, :], xa[p, i0:i1, :])
                t1 = t1_pool.tile([128, 4, W], dt_mm, tag="t1")
                for b, (o0, o1, i0, i1) in enumerate(blocks):
                    pa = psA.tile([o1 - o0, W], f32, tag="psA")
                    nc.tensor.matmul(pa[:], lv[sv][b][:], xv[: i1 - i0, b, :],
                                     start=True, stop=True)
                    copy_seg(t1, pa, o0, o1)

                # Stage B: halo'd row-tiles of t1^T via full-128 transposes.
                xh = xh_pool.tile([128, nb, W], dt_mm, tag="xh")
                for b, (o0, o1, i0, i1) in enumerate(blocks):
                    kb = i1 - i0
                    pb = psB.tile([128, W], f32, tag="psB")
                    for j in range(4):
                        nc.tensor.transpose(pb[:kb, 128 * j: 128 * (j + 1)],
                                            t1[:, j, i0:i1], ident[:])
                    copy(xh[:kb, b, :], pb[:kb, :])

                # Stage C: horizontal pass = vertical banded matmul on t1^T.
                t2 = t2_pool.tile([128, 4, W], dt_mm, tag="t2")
                for b, (o0, o1, i0, i1) in enumerate(blocks):
                    pc = psC.tile([o1 - o0, W], f32, tag="psC")
                    nc.tensor.matmul(pc[:], lh[sh][b][:], xh[: i1 - i0, b, :],
                                     start=True, stop=True)
                    copy_seg(t2, pc, o0, o1)

                # Stage D: transpose back to natural layout and store.
                ot = ot_pool.tile([128, 4, W], dt_mm, tag="ot")
                for m in range(4):
                    pd = psD.tile([128, W], f32, tag="psD")
                    for j in range(4):
                        nc.tensor.transpose(pd[:, 128 * j: 128 * (j + 1)],
                                            t2[:, j, 128 * m: 128 * (m + 1)],
                                            ident[:])
                    copy(ot[:, m, :], pd[:])
                    nc.sync.dma_start(oa[p, 128 * m: 128 * (m + 1), :],
                                      ot[:, m, :])

    nc.compile()
    return nc


def _prepare(weight, dt_name):
    kvs, khs = _factor_weight(weight)
    radius = max(max(_radius(k) for k in kvs), max(_radius(k) for k in khs))
    radius = min(radius, 15)
    blocks = _blocks(radius)

    # Dedupe per-channel band matrices.
    def uniq(ks):
        mats, idx = [], []
        for k in ks:
            CT = _conv_matrix(k).T.astype(np.float32)
            for i, m in enumerate(mats):
                if np.array_equal(m, CT):
                    idx.append(i)
                    break
            else:
                idx.append(len(mats))
                mats.append(CT)
        return mats, idx

    mv, ch2v = uniq(kvs)
    mh, ch2h = uniq(khs)

    consts = {"ident": np.eye(128, dtype=np.float32)}
    for s, m in enumerate(mv):
        for b, (o0, o1, i0, i1) in enumerate(blocks):
            consts[f"lv{s}_{b}"] = np.ascontiguousarray(m[i0:i1, o0:o1])
    for s, m in enumerate(mh):
        for b, (o0, o1, i0, i1) in enumerate(blocks):
            consts[f"lh{s}_{b}"] = np.ascontiguousarray(m[i0:i1, o0:o1])

    import concourse.mybir as mybir
    dt_mm = getattr(mybir.dt, dt_name)
    nc = _build_program(len(mv), len(mh), ch2v, ch2h, blocks, dt_mm)
    return nc, consts


def kernel(x, weight, _trace=False, _dt="float32"):
    key = (x.shape, weight.tobytes(), _dt)
    if key not in _cache:
        _cache.clear()
        _cache[key] = _prepare(weight, _dt)
    nc, consts = _cache[key]

    from concourse.bass_utils import run_bass_kernel_spmd

    n = x.shape[0]
    per = n // N_CORES
    in_maps = []
    for i in range(N_CORES):
        m = dict(consts)
        m["x"] = np.ascontiguousarray(
            x[i * per: (i + 1) * per].reshape(per * CH, H, W))
        in_maps.append(m)

    res = run_bass_kernel_spmd(nc, in_maps, list(range(N_CORES)),
                               trace=_trace)
    out = np.concatenate(
        [r["out"].reshape(per, CH, H, W) for r in res.results], axis=0)
    if _trace:
        kernel.last_exec_time_ns = res.exec_time_ns
        kernel.last_results = res
    return out
